# revision 9
# baseline (speedup 1.0000x reference)
"""Trainium2 Bass kernel for the mixed low-rank-expert DCN-v2 block (nn_DCN_51539607711).

Reference math (L=3 layers, E=4 experts, D=512, R=64, B=16384):
  x_{l+1} = sum_e x0 * (tanh(tanh(x_l V_e) C_e) U_e^T + b_l) * gate_e + x_l
The gate is softmax over a size-1 axis == exactly 1.0, so G never affects the
output. With gate == 1 the recurrence telescopes:
  x_{l+1} = x0 * (1 + sum_{i<=l} (A_i(x_i) + E*b_i)),
  A_i(x) = sum_e U_e tanh(C_e^T tanh(V_e^T x))
so the residual stream is carried as a single PSUM accumulator s = sum_i A_i
(fp32, accumulated by the PE across all experts AND layers), and each layer's
activation update is ONE fused DVE op per chunk:
  x_{l+1} = (s + c_l[d]) * x0,   c_l = 1 + E*cumsum(b)_l   (per-partition scalar)

Distribution: pure data-parallel over B across 8 cores (2048 rows/core),
weights replicated. Activations live feature-major (xT: [D, B]) so every
matmul contracts on the partition dim with zero on-device transposes; the
host pre-transposes x and pre-packs weights (experts packed in pairs to fill
all 128 partitions/output rows):
  v-step :  vT[pair]  = Vpair^T  @ xlT    (lhsT = Vpair [D,128], K=D in 4 chunks)
  cv-step:  cvT[pair] = blockdiag(C_e0,C_e1)^T @ vT[pair]   (K=128)
  ucv    :  s[mchunk] += Upair^T-packed @ cvT[pair]          (K=128, accum)
All matmul operands bf16 (fp32 PSUM accumulation); residual + output fp32.
Per-core B is processed in blocks of 512 columns so s (4 PSUM banks) plus
matmul transients (4 banks) exactly fill PSUM.
"""

import numpy as np
import ml_dtypes

import concourse.bacc as bacc
import concourse.tile as tile
from concourse import mybir
from concourse.bass_utils import run_bass_kernel_spmd

L, E, D, R, B = 3, 4, 512, 64, 16384
NCORES = 8
BC = B // NCORES          # batch columns per core (2048)
NB = 512                  # block of batch columns (one PSUM bank at fp32)
NBLK = BC // NB           # blocks per core
P = 128                   # partitions
KC = D // P               # contraction chunks over D (4)
NPAIR = E // 2            # expert pairs (2)

F32 = mybir.dt.float32
BF16 = mybir.dt.bfloat16
bf16 = ml_dtypes.bfloat16

_CACHE = {}


def _build_nc(bc=BC):
    """Build the per-core Bass/Tile kernel. Identical NEFF on all cores."""
    nblk = bc // NB
    nc = bacc.Bacc("TRN2", target_bir_lowering=False, debug=False,
                   num_devices=NCORES)

    xf_d = nc.dram_tensor("xf", [D, bc], F32, kind="ExternalInput")
    xb_d = nc.dram_tensor("xb", [D, bc], BF16, kind="ExternalInput")
    vw_d = nc.dram_tensor("vw", [P, L, NPAIR, KC, P], BF16, kind="ExternalInput")
    cw_d = nc.dram_tensor("cw", [P, L, NPAIR, P], BF16, kind="ExternalInput")
    uw_d = nc.dram_tensor("uw", [P, L, NPAIR, D], BF16, kind="ExternalInput")
    cb_d = nc.dram_tensor("cb", [P, L, KC], F32, kind="ExternalInput")
    out_d = nc.dram_tensor("out_t", [D, bc], F32, kind="ExternalOutput")

    Tanh = mybir.ActivationFunctionType.Tanh
    ADD = mybir.AluOpType.add
    MULT = mybir.AluOpType.mult

    with tile.TileContext(nc) as tc:
        with (
            tc.tile_pool(name="wpool", bufs=1) as wpool,
            tc.tile_pool(name="xpool", bufs=1) as xpool,
            tc.tile_pool(name="xl_pool", bufs=12) as xl_pool,
            tc.tile_pool(name="act_pool", bufs=8) as act_pool,
            tc.tile_pool(name="out_pool", bufs=4) as out_pool,
            tc.tile_pool(name="psum_s", bufs=4, space="PSUM") as psum_s,
            tc.tile_pool(name="psum_t", bufs=4, space="PSUM") as psum_t,
        ):
            # ---- persistent weights (one straight DMA each; host pre-packed) ----
            vw_s = wpool.tile([P, L, NPAIR, KC, P], BF16)
            nc.sync.dma_start(vw_s[:], vw_d[:])
            cw_s = wpool.tile([P, L, NPAIR, P], BF16)
            nc.sync.dma_start(cw_s[:], cw_d[:])
            uw_s = wpool.tile([P, L, NPAIR, D], BF16)
            nc.sync.dma_start(uw_s[:], uw_d[:])
            cb_s = wpool.tile([P, L, KC], F32)
            nc.sync.dma_start(cb_s[:], cb_d[:])

            xf_s = xpool.tile([P, KC, bc], F32)
            xb_s = xpool.tile([P, KC, bc], BF16)

            for b in range(nblk):
                bs = slice(b * NB, (b + 1) * NB)
                # per-block x loads (lets block 0 start before all x arrives)
                for k in range(KC):
                    nc.sync.dma_start(xb_s[:, k, bs], xb_d[k * P:(k + 1) * P, bs])
                for k in range(KC):
                    nc.sync.dma_start(xf_s[:, k, bs], xf_d[k * P:(k + 1) * P, bs])

                s_tiles = [psum_s.tile([P, NB], F32, name=f"s_{b}_{m}", tag="s")
                           for m in range(KC)]
                xl_cur = [xb_s[:, k, bs] for k in range(KC)]

                for l in range(L):
                    # v = tanh(Vpair^T @ xl), one [128, NB] tile per expert pair
                    vts = []
                    for p in range(NPAIR):
                        vps = psum_t.tile([P, NB], F32, name=f"vps_{b}_{l}_{p}",
                                          tag="pst")
                        for k in range(KC):
                            nc.tensor.matmul(vps[:], vw_s[:, l, p, k, :],
                                             xl_cur[k],
                                             start=(k == 0), stop=(k == KC - 1))
                        vt = act_pool.tile([P, NB], BF16, name=f"vt_{b}_{l}_{p}",
                                           tag="act")
                        nc.scalar.activation(vt[:], vps[:], Tanh)
                        vts.append(vt)
                    # cv = tanh(blockdiag(C)^T @ v)
                    cvts = []
                    for p in range(NPAIR):
                        cps = psum_t.tile([P, NB], F32, name=f"cps_{b}_{l}_{p}",
                                          tag="pst")
                        nc.tensor.matmul(cps[:], cw_s[:, l, p, :], vts[p][:],
                                         start=True, stop=True)
                        cvt = act_pool.tile([P, NB], BF16, name=f"cvt_{b}_{l}_{p}",
                                            tag="act")
                        nc.scalar.activation(cvt[:], cps[:], Tanh)
                        cvts.append(cvt)
                    # s[m] += Upacked^T @ cv   (accumulates across pairs AND
                    # layers; per-element has_written bits accumulate across
                    # sim "groups", so later layers bypass the group check
                    # and close each layer's group so the DVE may read s)
                    for p in range(NPAIR):
                        for m in range(KC):
                            nc.tensor.matmul(
                                s_tiles[m][:],
                                uw_s[:, l, p, m * P:(m + 1) * P],
                                cvts[p][:],
                                start=(l == 0 and p == 0),
                                stop=(p == NPAIR - 1),
                                skip_group_check=(l > 0),
                            )
                    # x_{l+1} = (s + c_l) * x0  — one fused DVE op per chunk
                    if l < L - 1:
                        nxt = []
                        for m in range(KC):
                            xln = xl_pool.tile([P, NB], BF16,
                                               name=f"xl_{b}_{l}_{m}", tag="xl")
                            nc.vector.scalar_tensor_tensor(
                                xln[:], s_tiles[m][:], cb_s[:, l, m:m + 1],
                                xf_s[:, m, bs], ADD, MULT)
                            nxt.append(xln)
                        xl_cur = [t[:] for t in nxt]
                    else:
                        for m in range(KC):
                            ot = out_pool.tile([P, NB], F32,
                                               name=f"ot_{b}_{m}", tag="ot")
                            nc.vector.scalar_tensor_tensor(
                                ot[:], s_tiles[m][:], cb_s[:, l, m:m + 1],
                                xf_s[:, m, bs], ADD, MULT)
                            nc.sync.dma_start(out_d[m * P:(m + 1) * P, bs], ot[:])

    nc.compile()
    return nc


def _prep_weights(U, V, C, bias):
    """Host-side packing into the exact SBUF layouts (see module docstring)."""
    VwH = np.empty([P, L, NPAIR, KC, P], dtype=bf16)
    UwH = np.empty([P, L, NPAIR, D], dtype=bf16)
    CwH = np.zeros([P, L, NPAIR, P], dtype=bf16)
    for l in range(L):
        for p in range(NPAIR):
            vpair = np.concatenate([V[l, 2 * p], V[l, 2 * p + 1]], axis=1)  # [D,128]
            VwH[:, l, p, :, :] = vpair.reshape(KC, P, P).transpose(1, 0, 2)
            upair = np.concatenate([U[l, 2 * p].T, U[l, 2 * p + 1].T], axis=0)  # [128,D]
            UwH[:, l, p, :] = upair
            CwH[:R, l, p, :R] = C[l, 2 * p]
            CwH[R:, l, p, R:] = C[l, 2 * p + 1]
    cb = 1.0 + E * np.cumsum(bias.astype(np.float32), axis=0)       # [L, D]
    cbH = np.ascontiguousarray(
        cb.reshape(L, KC, P).transpose(2, 0, 1)).astype(np.float32)  # [P, L, KC]
    return VwH, CwH, UwH, cbH


def _make_in_maps(x, U, V, C, G, bias):
    VwH, CwH, UwH, cbH = _prep_weights(np.asarray(U, np.float32),
                                       np.asarray(V, np.float32),
                                       np.asarray(C, np.float32),
                                       np.asarray(bias, np.float32))
    xT = np.asarray(x, np.float32).T                    # [D, B]
    in_maps = []
    for c in range(NCORES):
        xf = np.ascontiguousarray(xT[:, c * BC:(c + 1) * BC])
        in_maps.append({
            "xf": xf,
            "xb": xf.astype(bf16),
            "vw": VwH, "cw": CwH, "uw": UwH, "cb": cbH,
        })
    return in_maps


def _run(inputs, trace=False, **kw):
    key = "nc"
    if key not in _CACHE:
        _CACHE[key] = _build_nc()
    nc = _CACHE[key]
    in_maps = _make_in_maps(**inputs)
    res = run_bass_kernel_spmd(nc, in_maps, core_ids=list(range(NCORES)),
                               trace=trace, **kw)
    out = np.empty((B, D), np.float32)
    for c in range(NCORES):
        out[c * BC:(c + 1) * BC, :] = res.results[c]["out_t"].T
    return out, res


def kernel(**inputs) -> np.ndarray:
    out, _ = _run(inputs, trace=False)
    return out
